# revision 7
# baseline (speedup 1.0000x reference)
"""DAGNN forward on 8 Trainium2 NeuronCores.

Computation: a[:, :512] = x; for node i in topological (index) order:
a[:, i] = tanh(b[i] + sum_j W[i, j] * a[:, j]); y = a[:, 1536:2048].

Strategy (v2):
- Data-parallel over batch: 8 cores x 256 rows each. Activations stored
  transposed on-chip: aT[node, batch].
- Host computes DAG levels and packs WHOLE consecutive levels into
  128-node chunks (padded with zero rows), so every level lives in
  exactly one chunk: the serial chain is exactly one (matmul, tanh)
  link per level (89 links), no chunk-boundary double links.
- The batch is split into two 128-column halves forming two independent
  serial chains, interleaved on the ACT engine so one half's PE matmul
  + semaphore latency hides under the other half's tanh.
- Weight blocks per dst chunk: "old" off-diag blocks (sources >= 2
  chunks back; off the critical path), bd_pre/bd_last split of the
  previous chunk's block by source level (only bd_last - sources in the
  immediately preceding level - sits on the chain), and per-level
  in-chunk gather blocks (columns masked to the level).
- Optional dummy matmuls keep the PE busy so its DVFS p-state ramps to
  full clock.
"""

import sys

for _p in ("/opt/trn_rl_repo",):
    if _p not in sys.path:
        sys.path.append(_p)

import numpy as np

N_NODES = 2048
N_IN = 512
N_OUT = 512
DEG = 32
BATCH = 2048
NCORES = 8
BCORE = BATCH // NCORES  # 256
CH = 128
NCH_IN = N_IN // CH  # 4
HALF = BCORE // 2  # 128

# PE warmers: dummy matmuls emitted per chain slot to hold the tensor
# engine's p-state at full clock and hide the SBUF-access pipeline fill
# of the chain matmuls (issued back-to-back behind a busy PE). 0 disables.
N_DUMMY_A = 3  # between chainA and chainB
N_DUMMY_B = 3  # after chainB


def _prep(edge_src, edge_dst, edge_w, b):
    """Level-sort the DAG, pack whole levels into padded 128-node chunks,
    and pack dense 128x128 weight blocks."""
    edge_src = np.asarray(edge_src, dtype=np.int64)
    edge_dst = np.asarray(edge_dst, dtype=np.int64)
    edge_w = np.asarray(edge_w, dtype=np.float32)
    b = np.asarray(b, dtype=np.float32)

    src2 = edge_src.reshape(N_NODES - N_IN, DEG)
    level = np.zeros(N_NODES, np.int64)
    for i in range(N_IN, N_NODES):
        level[i] = level[src2[i - N_IN]].max() + 1
    n_lev = int(level.max())

    # nodes per level (orig ids, ascending)
    lev_nodes = [np.where(level == L)[0] for L in range(1, n_lev + 1)]

    # greedy pack consecutive whole levels into chunks of <= 128 nodes
    chunks_levels = []  # list of list of level indices (0-based into lev_nodes)
    cur, cur_n = [], 0
    for li, nodes in enumerate(lev_nodes):
        if cur_n + len(nodes) > CH:
            chunks_levels.append(cur)
            cur, cur_n = [], 0
        cur.append(li)
        cur_n += len(nodes)
    if cur:
        chunks_levels.append(cur)
    C = len(chunks_levels)

    # row coordinate space: rows 0..511 = input nodes (orig order),
    # then computed chunk c occupies rows 512 + c*128 .. +128 (padded).
    NROW = N_IN + C * CH
    row_of = np.full(N_NODES, -1, np.int64)
    row_of[:N_IN] = np.arange(N_IN)
    node_of_row = np.full(NROW, -1, np.int64)
    node_of_row[:N_IN] = np.arange(N_IN)
    chunk_levels_rows = []  # per chunk: list of (r0, r1) row ranges per level
    for c, lis in enumerate(chunks_levels):
        base = N_IN + c * CH
        off = 0
        lvls = []
        for li in lis:
            nodes = lev_nodes[li]
            r0 = off
            for nd in nodes:
                row_of[nd] = base + off
                node_of_row[base + off] = nd
                off += 1
            lvls.append((r0, off))
        chunk_levels_rows.append(lvls)

    # dense transposed weights in row coords: WT[src_row, dst_row]
    WT = np.zeros((NROW, NROW), np.float32)
    np.add.at(WT, (row_of[edge_src], row_of[edge_dst]), edge_w)

    # bias per chunk column [128, C]; padded rows get 0
    bias128 = np.zeros((CH, C), np.float32)
    for c in range(C):
        rows = node_of_row[N_IN + c * CH : N_IN + (c + 1) * CH]
        valid = rows >= 0
        bias128[valid, c] = b[rows[valid]]

    # level of each row within a computed chunk (for bd split)
    # chunk c's last level rows: chunk_levels_rows[c][-1]

    cols = []
    col = 0
    chunk_meta = []
    for t in range(C):
        dst0 = N_IN + t * CH
        D = WT[:, dst0 : dst0 + CH]
        old = []  # (coloff, src_tile)
        bd_pre = None
        bd_last = None
        nsrc = NCH_IN + t  # source tiles 0..nsrc-1
        for k in range(nsrc):
            blk = D[k * CH : (k + 1) * CH, :]
            if t >= 1 and k == nsrc - 1:
                # previous computed chunk: split entries by src level
                lr0, lr1 = chunk_levels_rows[t - 1][-1]
                pre = blk.copy()
                pre[lr0:lr1, :] = 0.0
                last = blk.copy()
                last[:lr0, :] = 0.0
                last[lr1:, :] = 0.0
                if pre.any():
                    cols.append(pre)
                    bd_pre = (col, k)
                    col += CH
                assert last.any(), "bd_last must be nonzero"
                cols.append(last)
                bd_last = (col, k)
                col += CH
            else:
                if not blk.any():
                    continue
                cols.append(blk)
                old.append((col, k))
                col += CH
        # in-chunk gathers per level gi >= 1
        Din = WT[dst0 : dst0 + CH, dst0 : dst0 + CH]
        gth = []
        lvls = chunk_levels_rows[t]
        for gi in range(1, len(lvls)):
            r0, r1 = lvls[gi]
            blk = np.zeros((CH, CH), np.float32)
            blk[:, r0:r1] = Din[:, r0:r1]
            assert blk.any(), "in-chunk gather must be nonzero"
            cols.append(blk)
            gth.append((col,))
            col += CH
        chunk_meta.append(dict(old=old, bd_pre=bd_pre, bd_last=bd_last, gth=gth))

    wflat = np.ascontiguousarray(np.concatenate(cols, axis=1))  # [128, col]
    out_rows = row_of[np.arange(N_NODES - N_OUT, N_NODES)] - N_IN

    return dict(
        C=C,
        chunk_levels_rows=chunk_levels_rows,
        chunk_meta=chunk_meta,
        wflat=wflat,
        bias128=bias128,
        out_rows=out_rows,
        row_of=row_of,
    )


def _emulate(prep, xT):
    """Numpy emulation of the exact block scheme (per core). xT: [512, B]."""
    B = xT.shape[1]
    C = prep["C"]
    wflat = prep["wflat"]
    bias = prep["bias128"]
    NROW = N_IN + C * CH
    aT = np.zeros((NROW, B), np.float32)
    aT[:N_IN] = xT
    for t in range(C):
        meta = prep["chunk_meta"][t]
        dst0 = N_IN + t * CH
        psum = np.zeros((CH, B), np.float32)
        mms = list(meta["old"])
        if meta["bd_pre"]:
            mms.append(meta["bd_pre"])
        if meta["bd_last"]:
            mms.append(meta["bd_last"])
        for coloff, k in mms:
            blk = wflat[:, coloff : coloff + CH]
            psum += blk.T @ aT[k * CH : (k + 1) * CH]
        lvls = prep["chunk_levels_rows"][t]
        for gi, (r0, r1) in enumerate(lvls):
            if gi > 0:
                (coloff,) = meta["gth"][gi - 1]
                blk = wflat[:, coloff : coloff + CH]
                psum += blk.T @ aT[dst0 : dst0 + CH]
            # full-tile tanh (idempotent rewrite like the kernel)
            aT[dst0 : dst0 + CH] = np.tanh(psum + bias[:, t : t + 1])
    return aT[N_IN:]  # [C*128, B]


def _build_program(prep):
    """Build the Bass/Tile program (identical for all 8 cores)."""
    import concourse.bacc as bacc
    import concourse.tile as tile
    from concourse import mybir

    f32 = mybir.dt.float32
    f16 = mybir.dt.float16
    nc = bacc.Bacc(
        "TRN2",
        target_bir_lowering=False,
        debug=False,
        enable_asserts=False,
        num_devices=NCORES,
    )
    C = prep["C"]
    wflat = prep["wflat"]
    F = wflat.shape[1]
    meta = prep["chunk_meta"]
    lvls_all = prep["chunk_levels_rows"]

    xT_d = nc.dram_tensor("xT", [N_IN, BCORE], f16, kind="ExternalInput").ap()
    w_d = nc.dram_tensor("wflat", [CH, F], f16, kind="ExternalInput").ap()
    b_d = nc.dram_tensor("bias", [CH, C], f32, kind="ExternalInput").ap()
    out_d = nc.dram_tensor("outT", [C * CH, BCORE], f16, kind="ExternalOutput").ap()

    # per-chunk W tile span (cols) within wflat
    chunk_w0, chunk_w1 = [], []
    for t in range(C):
        offs = [c for c, _ in meta[t]["old"]]
        if meta[t]["bd_pre"]:
            offs.append(meta[t]["bd_pre"][0])
        if meta[t]["bd_last"]:
            offs.append(meta[t]["bd_last"][0])
        offs += [g[0] for g in meta[t]["gth"]]
        chunk_w0.append(min(offs))
        chunk_w1.append(max(offs) + CH)

    with tile.TileContext(nc) as tc:
        with (
            tc.tile_pool(name="aT", bufs=1) as aT_pool,
            tc.tile_pool(name="wpool", bufs=3) as w_pool,
            tc.tile_pool(name="small", bufs=1) as small_pool,
            tc.tile_pool(name="psum", bufs=3, space="PSUM") as psum_pool,
            tc.tile_pool(name="dpsum", bufs=1, space="PSUM") as dpsum_pool,
        ):
            # input activations: one wide tile, single DMA
            xTall = aT_pool.tile([CH, NCH_IN * BCORE], f16, tag="xTall")
            aTc = [
                aT_pool.tile([CH, BCORE], f16, tag=f"aT{c}", name=f"aT{c}")
                for c in range(C)
            ]
            bias_t = small_pool.tile([CH, C], f32, tag="bias")
            scratch = small_pool.tile([CH, 1], f32, tag="scratch")
            dummy_ps = dpsum_pool.tile([32, 64], f32, tag="dummy")

            def rhs(src_tile, lo, hi):
                if src_tile < NCH_IN:
                    base = src_tile * BCORE
                    return xTall[:, base + lo : base + hi]
                return aTc[src_tile - NCH_IN][:, lo:hi]

            w_tiles = [None] * C

            def wdma(t):
                wid = chunk_w1[t] - chunk_w0[t]
                w_tiles[t] = w_pool.tile([CH, wid], f16, tag="w", name=f"w{t}")
                if t == 0:
                    # split so the first matmuls wait only on the off-diag
                    # blocks, not the whole span
                    split = (
                        max(c for c, _ in meta[0]["old"]) + CH - chunk_w0[0]
                    )
                    nc.sync.dma_start(
                        out=w_tiles[0][:, :split],
                        in_=w_d[:, chunk_w0[0] : chunk_w0[0] + split],
                    )
                    if split < wid:
                        nc.sync.dma_start(
                            out=w_tiles[0][:, split:],
                            in_=w_d[:, chunk_w0[0] + split : chunk_w1[0]],
                        )
                    return
                nc.sync.dma_start(
                    out=w_tiles[t][:], in_=w_d[:, chunk_w0[t] : chunk_w1[t]]
                )

            # --- matmul bookkeeping: start/stop flags -------------------
            n_full = [
                len(meta[t]["old"]) + (1 if meta[t]["bd_pre"] else 0)
                for t in range(C)
            ]
            n_half = [
                (1 if meta[t]["bd_last"] else 0) + len(meta[t]["gth"])
                for t in range(C)
            ]
            cnt_full = [0] * C
            cnt_half = [[0, 0] for _ in range(C)]
            psum_t = [None] * C

            def get_psum(t):
                if psum_t[t] is None:
                    psum_t[t] = psum_pool.tile(
                        [CH, BCORE], f32, tag="psum", name=f"psum{t}"
                    )
                return psum_t[t]

            def emit_full(t, coloff, src_tile):
                ps = get_psum(t)
                first = cnt_full[t] == 0
                last = n_half[t] == 0 and cnt_full[t] == n_full[t] - 1
                a = coloff - chunk_w0[t]
                nc.tensor.matmul(
                    ps[:, :],
                    w_tiles[t][:, a : a + CH],
                    rhs(src_tile, 0, BCORE),
                    start=first,
                    stop=last,
                )
                cnt_full[t] += 1

            def emit_half(t, coloff, src_tile, h):
                ps = get_psum(t)
                assert cnt_full[t] == n_full[t], (
                    f"half mm before all full mms: chunk {t}"
                )
                last = cnt_half[t][h] == n_half[t] - 1
                a = coloff - chunk_w0[t]
                lo, hi = h * HALF, (h + 1) * HALF
                nc.tensor.matmul(
                    ps[:, lo:hi],
                    w_tiles[t][:, a : a + CH],
                    rhs(src_tile, lo, hi),
                    start=False,
                    stop=last,
                )
                cnt_half[t][h] += 1

            def emit_dummies(n):
                for _ in range(n):
                    nc.tensor.matmul(
                        dummy_ps[:, :],
                        xTall[:, 0:32],
                        xTall[:, 0:64],
                        start=True,
                        stop=True,
                    )

            def emit_act(t, gi, h):
                lo, hi = h * HALF, (h + 1) * HALF
                nc.scalar.activation(
                    aTc[t][:, lo:hi],
                    psum_t[t][:, lo:hi],
                    mybir.ActivationFunctionType.Tanh,
                    bias=bias_t[:, t : t + 1],
                )

            # --- prologue ----------------------------------------------
            wdma(0)
            # xT as one DMA: dst free dims [chunk, batch] <-> DRAM rows
            nc.sync.dma_start(
                out=xTall[:].rearrange("p (c b) -> p c b", c=NCH_IN),
                in_=xT_d.rearrange("(c p) b -> p c b", c=NCH_IN),
            )
            nc.sync.dma_start(out=bias_t[:], in_=b_d[:])
            # preload the tanh table during the prologue
            nc.scalar.activation(
                scratch[:], bias_t[:, 0:1], mybir.ActivationFunctionType.Tanh
            )
            wdma(1)
            wdma(2)
            # chunk 0 full accumulation
            for coloff, k in meta[0]["old"]:
                emit_full(0, coloff, k)

            # --- main walk ---------------------------------------------
            for t in range(C):
                if t + 3 < C:
                    wdma(t + 3)
                lvls = lvls_all[t]
                G = len(lvls)
                # fillers: next chunk's old mms, woven across this chunk's
                # level slots (they only need tiles <= t-1, all final).
                pending = list(meta[t + 1]["old"]) if t + 1 < C else []
                n_slots = 2 * G  # a filler window after each half's chain mm
                n_fill = len(pending)
                fil_acc = 0

                def weave():
                    nonlocal fil_acc
                    fil_acc += n_fill
                    take = fil_acc // n_slots
                    fil_acc -= take * n_slots
                    for _ in range(take):
                        if pending:
                            coloff, k = pending.pop(0)
                            emit_full(t + 1, coloff, k)
                for gi in range(G):
                    # chain mm half A
                    if gi == 0:
                        if t >= 1:
                            emit_half(t, meta[t]["bd_last"][0], NCH_IN + t - 1, 0)
                    else:
                        (coloff,) = meta[t]["gth"][gi - 1]
                        emit_half(t, coloff, NCH_IN + t, 0)
                    emit_act(t, gi, 0)
                    weave()
                    emit_dummies(N_DUMMY_A)
                    # chain mm half B
                    if gi == 0:
                        if t >= 1:
                            emit_half(t, meta[t]["bd_last"][0], NCH_IN + t - 1, 1)
                    else:
                        (coloff,) = meta[t]["gth"][gi - 1]
                        emit_half(t, coloff, NCH_IN + t, 1)
                    emit_act(t, gi, 1)
                    if gi == G - 2 and t + 1 < C and meta[t + 1]["bd_pre"]:
                        # bd_pre depends on acts of this slot (levels < last)
                        emit_full(t + 1, meta[t + 1]["bd_pre"][0], NCH_IN + t)
                    weave()
                    emit_dummies(N_DUMMY_B)
                while pending:
                    coloff, k = pending.pop(0)
                    emit_full(t + 1, coloff, k)
                nc.sync.dma_start(
                    out=out_d[t * CH : (t + 1) * CH, :], in_=aTc[t][:]
                )

    nc.compile()
    return nc


def _make_in_maps(prep, x):
    x = np.asarray(x, dtype=np.float32)
    in_maps = []
    for r in range(NCORES):
        xr = x[r * BCORE : (r + 1) * BCORE]  # [256, 512]
        in_maps.append(
            {
                "xT": np.ascontiguousarray(xr.T).astype(np.float16),
                "wflat": prep["wflat"].astype(np.float16),
                "bias": prep["bias128"],
            }
        )
    return in_maps


def _assemble(prep, results):
    out_rows = prep["out_rows"]
    y = np.empty((BATCH, N_OUT), np.float32)
    for r in range(NCORES):
        outT = results[r]["outT"].astype(np.float32)
        y[r * BCORE : (r + 1) * BCORE, :] = outT[out_rows, :].T
    return y


def kernel(x, edge_w, b, edge_src, edge_dst, n_out, _trace=False):
    n_out = int(n_out)
    assert n_out == N_OUT, f"hardcoded for n_out={N_OUT}, got {n_out}"
    x = np.asarray(x, dtype=np.float32)
    assert x.shape == (BATCH, N_IN)

    from concourse.bass_utils import run_bass_kernel_spmd

    prep = _prep(edge_src, edge_dst, edge_w, b)
    nc = _build_program(prep)
    in_maps = _make_in_maps(prep, x)
    res = run_bass_kernel_spmd(
        nc, in_maps, core_ids=list(range(NCORES)), trace=_trace
    )
    y = _assemble(prep, res.results)
    if _trace:
        kernel._last_exec_time_ns = res.exec_time_ns
        kernel._last_results = res
    return y


if __name__ == "__main__":
    # host-side emulation check against the jax reference
    sys.path.insert(0, "/root/problem")
    import os

    os.environ.setdefault("JAX_PLATFORMS", "cpu")
    import reference

    inputs = {k: np.asarray(v) for k, v in reference.setup_inputs().items()}
    prep = _prep(
        inputs["edge_src"], inputs["edge_dst"], inputs["edge_w"], inputs["b"]
    )
    print(
        f"C={prep['C']} chunks, F={prep['wflat'].shape[1]} cols, "
        f"levels/chunk={[len(l) for l in prep['chunk_levels_rows']]}"
    )
    expected = np.asarray(reference.reference(**reference.setup_inputs()))
    xT = inputs["x"][:8].T.astype(np.float32)
    aT = _emulate(prep, xT)
    got = aT[prep["out_rows"], :].T
    err = np.abs(got - expected[:8]).max()
    rel = err / np.abs(expected[:8]).max()
    print(f"emulation absmax err {err:.3e}  rel {rel:.3e}")


# revision 13
# speedup vs baseline: 1.4428x; 1.4428x over previous
"""DAGNN forward on 8 Trainium2 NeuronCores.

Computation: a[:, :512] = x; for node i in topological (index) order:
a[:, i] = tanh(b[i] + sum_j W[i, j] * a[:, j]); y = a[:, 1536:2048].

Strategy (v2):
- Data-parallel over batch: 8 cores x 256 rows each. Activations stored
  transposed on-chip: aT[node, batch].
- Host computes DAG levels and packs WHOLE consecutive levels into
  128-node chunks (padded with zero rows), so every level lives in
  exactly one chunk: the serial chain is exactly one (matmul, tanh)
  link per level (89 links), no chunk-boundary double links.
- The batch is split into two 128-column halves forming two independent
  serial chains, interleaved on the ACT engine so one half's PE matmul
  + semaphore latency hides under the other half's tanh.
- Weight blocks per dst chunk: "old" off-diag blocks (sources >= 2
  chunks back; off the critical path), bd_pre/bd_last split of the
  previous chunk's block by source level (only bd_last - sources in the
  immediately preceding level - sits on the chain), and per-level
  in-chunk gather blocks (columns masked to the level).
- Optional dummy matmuls keep the PE busy so its DVFS p-state ramps to
  full clock.
"""

import sys

for _p in ("/opt/trn_rl_repo",):
    if _p not in sys.path:
        sys.path.append(_p)

import numpy as np

N_NODES = 2048
N_IN = 512
N_OUT = 512
DEG = 32
BATCH = 2048
NCORES = 8
BCORE = BATCH // NCORES  # 256
CH = 128
NCH_IN = N_IN // CH  # 4
HALF = BCORE // 2  # 128



def _prep(edge_src, edge_dst, edge_w, b):
    """Level-sort the DAG, pack whole levels into padded 128-node chunks,
    and pack dense 128x128 weight blocks."""
    edge_src = np.asarray(edge_src, dtype=np.int64)
    edge_dst = np.asarray(edge_dst, dtype=np.int64)
    edge_w = np.asarray(edge_w, dtype=np.float32)
    b = np.asarray(b, dtype=np.float32)

    src2 = edge_src.reshape(N_NODES - N_IN, DEG)
    level = np.zeros(N_NODES, np.int64)
    for i in range(N_IN, N_NODES):
        level[i] = level[src2[i - N_IN]].max() + 1
    n_lev = int(level.max())

    # nodes per level (orig ids, ascending)
    lev_nodes = [np.where(level == L)[0] for L in range(1, n_lev + 1)]

    # greedy pack consecutive whole levels into chunks of <= 128 nodes
    chunks_levels = []  # list of list of level indices (0-based into lev_nodes)
    cur, cur_n = [], 0
    for li, nodes in enumerate(lev_nodes):
        if cur_n + len(nodes) > CH:
            chunks_levels.append(cur)
            cur, cur_n = [], 0
        cur.append(li)
        cur_n += len(nodes)
    if cur:
        chunks_levels.append(cur)
    C = len(chunks_levels)

    # row coordinate space: rows 0..511 = input nodes (orig order),
    # then computed chunk c occupies rows 512 + c*128 .. +128 (padded).
    NROW = N_IN + C * CH
    row_of = np.full(N_NODES, -1, np.int64)
    row_of[:N_IN] = np.arange(N_IN)
    node_of_row = np.full(NROW, -1, np.int64)
    node_of_row[:N_IN] = np.arange(N_IN)
    chunk_levels_rows = []  # per chunk: list of (r0, r1) row ranges per level
    for c, lis in enumerate(chunks_levels):
        base = N_IN + c * CH
        off = 0
        lvls = []
        for li in lis:
            nodes = lev_nodes[li]
            r0 = off
            for nd in nodes:
                row_of[nd] = base + off
                node_of_row[base + off] = nd
                off += 1
            lvls.append((r0, off))
        chunk_levels_rows.append(lvls)

    # dense transposed weights in row coords: WT[src_row, dst_row]
    WT = np.zeros((NROW, NROW), np.float32)
    np.add.at(WT, (row_of[edge_src], row_of[edge_dst]), edge_w)

    # bias per chunk column [128, C]; padded rows get 0
    bias128 = np.zeros((CH, C), np.float32)
    for c in range(C):
        rows = node_of_row[N_IN + c * CH : N_IN + (c + 1) * CH]
        valid = rows >= 0
        bias128[valid, c] = b[rows[valid]]

    # level of each row within a computed chunk (for bd split)
    # chunk c's last level rows: chunk_levels_rows[c][-1]

    cols = []
    col = 0
    chunk_meta = []
    for t in range(C):
        dst0 = N_IN + t * CH
        D = WT[:, dst0 : dst0 + CH]
        old = []  # (coloff, src_tile)
        bd_pre = None
        bd_last = None
        nsrc = NCH_IN + t  # source tiles 0..nsrc-1
        for k in range(nsrc):
            blk = D[k * CH : (k + 1) * CH, :]
            if t >= 1 and k == nsrc - 1:
                # previous computed chunk: split entries by src level
                lr0, lr1 = chunk_levels_rows[t - 1][-1]
                pre = blk.copy()
                pre[lr0:lr1, :] = 0.0
                last = blk.copy()
                last[:lr0, :] = 0.0
                last[lr1:, :] = 0.0
                if pre.any():
                    cols.append(pre)
                    bd_pre = (col, k)
                    col += CH
                assert last.any(), "bd_last must be nonzero"
                cols.append(last)
                bd_last = (col, k)
                col += CH
            else:
                if not blk.any():
                    continue
                cols.append(blk)
                old.append((col, k))
                col += CH
        # in-chunk gathers per level gi >= 1
        Din = WT[dst0 : dst0 + CH, dst0 : dst0 + CH]
        gth = []
        lvls = chunk_levels_rows[t]
        for gi in range(1, len(lvls)):
            r0, r1 = lvls[gi]
            blk = np.zeros((CH, CH), np.float32)
            blk[:, r0:r1] = Din[:, r0:r1]
            assert blk.any(), "in-chunk gather must be nonzero"
            cols.append(blk)
            gth.append((col,))
            col += CH
        chunk_meta.append(dict(old=old, bd_pre=bd_pre, bd_last=bd_last, gth=gth))

    wflat = np.ascontiguousarray(np.concatenate(cols, axis=1))  # [128, col]
    out_rows = row_of[np.arange(N_NODES - N_OUT, N_NODES)] - N_IN

    return dict(
        C=C,
        chunk_levels_rows=chunk_levels_rows,
        chunk_meta=chunk_meta,
        wflat=wflat,
        bias128=bias128,
        out_rows=out_rows,
        row_of=row_of,
    )


def _emulate(prep, xT):
    """Numpy emulation of the exact block scheme (per core). xT: [512, B]."""
    B = xT.shape[1]
    C = prep["C"]
    wflat = prep["wflat"]
    bias = prep["bias128"]
    NROW = N_IN + C * CH
    aT = np.zeros((NROW, B), np.float32)
    aT[:N_IN] = xT
    for t in range(C):
        meta = prep["chunk_meta"][t]
        dst0 = N_IN + t * CH
        psum = np.zeros((CH, B), np.float32)
        mms = list(meta["old"])
        if meta["bd_pre"]:
            mms.append(meta["bd_pre"])
        if meta["bd_last"]:
            mms.append(meta["bd_last"])
        for coloff, k in mms:
            blk = wflat[:, coloff : coloff + CH]
            psum += blk.T @ aT[k * CH : (k + 1) * CH]
        lvls = prep["chunk_levels_rows"][t]
        for gi, (r0, r1) in enumerate(lvls):
            if gi > 0:
                (coloff,) = meta["gth"][gi - 1]
                blk = wflat[:, coloff : coloff + CH]
                psum += blk.T @ aT[dst0 : dst0 + CH]
            # full-tile tanh (idempotent rewrite like the kernel)
            aT[dst0 : dst0 + CH] = np.tanh(psum + bias[:, t : t + 1])
    return aT[N_IN:]  # [C*128, B]


def _build_program(prep):
    """Build the Bass/Tile program (identical for all 8 cores)."""
    import concourse.bacc as bacc
    import concourse.tile as tile
    from concourse import mybir

    f32 = mybir.dt.float32
    f16 = mybir.dt.float16
    nc = bacc.Bacc(
        "TRN2",
        target_bir_lowering=False,
        debug=False,
        enable_asserts=False,
        num_devices=NCORES,
    )
    C = prep["C"]
    wflat = prep["wflat"]
    F = wflat.shape[1]
    meta = prep["chunk_meta"]
    lvls_all = prep["chunk_levels_rows"]

    # xT pre-laid-out host-side in SBUF layout: [128, n_in_chunks * batch]
    xT_d = nc.dram_tensor(
        "xT", [CH, NCH_IN * BCORE], f16, kind="ExternalInput"
    ).ap()
    w_d = nc.dram_tensor("wflat", [CH, F], f16, kind="ExternalInput").ap()
    b_d = nc.dram_tensor("bias", [CH, C], f32, kind="ExternalInput").ap()
    out_d = nc.dram_tensor("outT", [C * CH, BCORE], f16, kind="ExternalOutput").ap()

    # per-chunk W tile span (cols) within wflat
    chunk_w0, chunk_w1 = [], []
    for t in range(C):
        offs = [c for c, _ in meta[t]["old"]]
        if meta[t]["bd_pre"]:
            offs.append(meta[t]["bd_pre"][0])
        if meta[t]["bd_last"]:
            offs.append(meta[t]["bd_last"][0])
        offs += [g[0] for g in meta[t]["gth"]]
        chunk_w0.append(min(offs))
        chunk_w1.append(max(offs) + CH)

    with tile.TileContext(nc) as tc:
        with (
            tc.tile_pool(name="aT", bufs=1) as aT_pool,
            tc.tile_pool(name="wpool", bufs=3) as w_pool,
            tc.tile_pool(name="small", bufs=1) as small_pool,
            tc.tile_pool(name="psum", bufs=3, space="PSUM") as psum_pool,
        ):
            # input activations: one wide tile, single DMA
            xTall = aT_pool.tile([CH, NCH_IN * BCORE], f16, tag="xTall")
            aTc = [
                aT_pool.tile([CH, BCORE], f16, tag=f"aT{c}", name=f"aT{c}")
                for c in range(C)
            ]
            bias_t = small_pool.tile([CH, C], f32, tag="bias")
            scratch = small_pool.tile([CH, 1], f32, tag="scratch")

            def rhs(src_tile, lo, hi):
                if src_tile < NCH_IN:
                    base = src_tile * BCORE
                    return xTall[:, base + lo : base + hi]
                return aTc[src_tile - NCH_IN][:, lo:hi]

            w_tiles = [None] * C

            def wdma(t):
                wid = chunk_w1[t] - chunk_w0[t]
                w_tiles[t] = w_pool.tile([CH, wid], f16, tag="w", name=f"w{t}")
                if t == 0:
                    # split so the first matmuls wait only on the off-diag
                    # blocks, not the whole span
                    split = (
                        max(c for c, _ in meta[0]["old"]) + CH - chunk_w0[0]
                    )
                    nc.sync.dma_start(
                        out=w_tiles[0][:, :split],
                        in_=w_d[:, chunk_w0[0] : chunk_w0[0] + split],
                    )
                    if split < wid:
                        nc.sync.dma_start(
                            out=w_tiles[0][:, split:],
                            in_=w_d[:, chunk_w0[0] + split : chunk_w1[0]],
                        )
                    return
                nc.sync.dma_start(
                    out=w_tiles[t][:], in_=w_d[:, chunk_w0[t] : chunk_w1[t]]
                )

            # --- matmul bookkeeping: start/stop flags -------------------
            n_mms = [
                len(meta[t]["old"])
                + (1 if meta[t]["bd_pre"] else 0)
                + (1 if meta[t]["bd_last"] else 0)
                + len(meta[t]["gth"])
                for t in range(C)
            ]
            cnt_mm = [0] * C
            psum_t = [None] * C

            def get_psum(t):
                if psum_t[t] is None:
                    psum_t[t] = psum_pool.tile(
                        [CH, BCORE], f32, tag="psum", name=f"psum{t}"
                    )
                return psum_t[t]

            def emit_mm(t, coloff, src_tile):
                ps = get_psum(t)
                first = cnt_mm[t] == 0
                last = cnt_mm[t] == n_mms[t] - 1
                a = coloff - chunk_w0[t]
                nc.tensor.matmul(
                    ps[:, :],
                    w_tiles[t][:, a : a + CH],
                    rhs(src_tile, 0, BCORE),
                    start=first,
                    stop=last,
                )
                cnt_mm[t] += 1

            def emit_act(t, gi):
                nc.scalar.activation(
                    aTc[t][:, :],
                    psum_t[t][:, :],
                    mybir.ActivationFunctionType.Tanh,
                    bias=bias_t[:, t : t + 1],
                )

            # --- prologue ----------------------------------------------
            wdma(0)
            # xT: one contiguous DMA (host already produced SBUF layout)
            nc.sync.dma_start(out=xTall[:], in_=xT_d[:, :])
            nc.sync.dma_start(out=bias_t[:], in_=b_d[:])
            # preload the tanh table during the prologue
            nc.scalar.activation(
                scratch[:], bias_t[:, 0:1], mybir.ActivationFunctionType.Tanh
            )
            wdma(1)
            wdma(2)
            # chunk 0 full accumulation
            for coloff, k in meta[0]["old"]:
                emit_mm(0, coloff, k)

            # --- main walk ---------------------------------------------
            for t in range(C):
                if t + 3 < C:
                    wdma(t + 3)
                lvls = lvls_all[t]
                G = len(lvls)
                # fillers: next chunk's old mms, woven across this chunk's
                # level slots (they only need tiles <= t-1, all final).
                pending = list(meta[t + 1]["old"]) if t + 1 < C else []
                n_slots = G
                n_fill = len(pending)
                fil_acc = 0
                for gi in range(G):
                    # chain mm
                    if gi == 0:
                        if t >= 1:
                            emit_mm(t, meta[t]["bd_last"][0], NCH_IN + t - 1)
                    else:
                        (coloff,) = meta[t]["gth"][gi - 1]
                        emit_mm(t, coloff, NCH_IN + t)
                    emit_act(t, gi)
                    if gi == G - 2 and t + 1 < C and meta[t + 1]["bd_pre"]:
                        # bd_pre depends on acts of this slot (levels < last)
                        emit_mm(t + 1, meta[t + 1]["bd_pre"][0], NCH_IN + t)
                    # Bresenham-spread fillers; they run during this level's
                    # act and must clear before the next chain mm's sem
                    fil_acc += n_fill
                    take = fil_acc // n_slots
                    fil_acc -= take * n_slots
                    for _ in range(take):
                        if pending:
                            coloff, k = pending.pop(0)
                            emit_mm(t + 1, coloff, k)
                while pending:
                    coloff, k = pending.pop(0)
                    emit_mm(t + 1, coloff, k)
                nc.sync.dma_start(
                    out=out_d[t * CH : (t + 1) * CH, :], in_=aTc[t][:]
                )

    nc.compile()
    return nc


def _make_in_maps(prep, x):
    x = np.asarray(x, dtype=np.float32)
    in_maps = []
    wflat16 = prep["wflat"].astype(np.float16)
    for r in range(NCORES):
        xr = x[r * BCORE : (r + 1) * BCORE]  # [256, 512]
        xT = xr.T.astype(np.float16)  # [512, 256]
        # SBUF layout: partition p, free = chunk-major: [128, 4*256]
        xT = np.ascontiguousarray(
            xT.reshape(NCH_IN, CH, BCORE).transpose(1, 0, 2).reshape(
                CH, NCH_IN * BCORE
            )
        )
        in_maps.append({"xT": xT, "wflat": wflat16, "bias": prep["bias128"]})
    return in_maps


def _assemble(prep, results):
    out_rows = prep["out_rows"]
    y = np.empty((BATCH, N_OUT), np.float32)
    for r in range(NCORES):
        outT = results[r]["outT"].astype(np.float32)
        y[r * BCORE : (r + 1) * BCORE, :] = outT[out_rows, :].T
    return y


def kernel(x, edge_w, b, edge_src, edge_dst, n_out, _trace=False):
    n_out = int(n_out)
    assert n_out == N_OUT, f"hardcoded for n_out={N_OUT}, got {n_out}"
    x = np.asarray(x, dtype=np.float32)
    assert x.shape == (BATCH, N_IN)

    from concourse.bass_utils import run_bass_kernel_spmd

    prep = _prep(edge_src, edge_dst, edge_w, b)
    nc = _build_program(prep)
    in_maps = _make_in_maps(prep, x)
    res = run_bass_kernel_spmd(
        nc, in_maps, core_ids=list(range(NCORES)), trace=_trace
    )
    y = _assemble(prep, res.results)
    if _trace:
        kernel._last_exec_time_ns = res.exec_time_ns
        kernel._last_results = res
    return y


if __name__ == "__main__":
    # host-side emulation check against the jax reference
    sys.path.insert(0, "/root/problem")
    import os

    os.environ.setdefault("JAX_PLATFORMS", "cpu")
    import reference

    inputs = {k: np.asarray(v) for k, v in reference.setup_inputs().items()}
    prep = _prep(
        inputs["edge_src"], inputs["edge_dst"], inputs["edge_w"], inputs["b"]
    )
    print(
        f"C={prep['C']} chunks, F={prep['wflat'].shape[1]} cols, "
        f"levels/chunk={[len(l) for l in prep['chunk_levels_rows']]}"
    )
    expected = np.asarray(reference.reference(**reference.setup_inputs()))
    xT = inputs["x"][:8].T.astype(np.float32)
    aT = _emulate(prep, xT)
    got = aT[prep["out_rows"], :].T
    err = np.abs(got - expected[:8]).max()
    rel = err / np.abs(expected[:8]).max()
    print(f"emulation absmax err {err:.3e}  rel {rel:.3e}")


# revision 15
# speedup vs baseline: 1.5123x; 1.0481x over previous
"""DAGNN forward on 8 Trainium2 NeuronCores.

Computation: a[:, :512] = x; for node i in topological (index) order:
a[:, i] = tanh(b[i] + sum_j W[i, j] * a[:, j]); y = a[:, 1536:2048].

Strategy (v2):
- Data-parallel over batch: 8 cores x 256 rows each. Activations stored
  transposed on-chip: aT[node, batch].
- Host computes DAG levels and packs WHOLE consecutive levels into
  128-node chunks (padded with zero rows), so every level lives in
  exactly one chunk: the serial chain is exactly one (matmul, tanh)
  link per level (89 links), no chunk-boundary double links.
- The batch is split into two 128-column halves forming two independent
  serial chains, interleaved on the ACT engine so one half's PE matmul
  + semaphore latency hides under the other half's tanh.
- Weight blocks per dst chunk: "old" off-diag blocks (sources >= 2
  chunks back; off the critical path), bd_pre/bd_last split of the
  previous chunk's block by source level (only bd_last - sources in the
  immediately preceding level - sits on the chain), and per-level
  in-chunk gather blocks (columns masked to the level).
- Optional dummy matmuls keep the PE busy so its DVFS p-state ramps to
  full clock.
"""

import sys

for _p in ("/opt/trn_rl_repo",):
    if _p not in sys.path:
        sys.path.append(_p)

import numpy as np

N_NODES = 2048
N_IN = 512
N_OUT = 512
DEG = 32
BATCH = 2048
NCORES = 8
BCORE = BATCH // NCORES  # 256
CH = 128
NCH_IN = N_IN // CH  # 4
HALF = BCORE // 2  # 128



def _prep(edge_src, edge_dst, edge_w, b):
    """Level-sort the DAG, pack whole levels into padded 128-node chunks,
    and pack dense 128x128 weight blocks."""
    edge_src = np.asarray(edge_src, dtype=np.int64)
    edge_dst = np.asarray(edge_dst, dtype=np.int64)
    edge_w = np.asarray(edge_w, dtype=np.float32)
    b = np.asarray(b, dtype=np.float32)

    src2 = edge_src.reshape(N_NODES - N_IN, DEG)
    level = np.zeros(N_NODES, np.int64)
    for i in range(N_IN, N_NODES):
        level[i] = level[src2[i - N_IN]].max() + 1
    n_lev = int(level.max())

    # nodes per level (orig ids, ascending)
    lev_nodes = [np.where(level == L)[0] for L in range(1, n_lev + 1)]

    # greedy pack consecutive whole levels into chunks of <= 128 nodes
    chunks_levels = []  # list of list of level indices (0-based into lev_nodes)
    cur, cur_n = [], 0
    for li, nodes in enumerate(lev_nodes):
        if cur_n + len(nodes) > CH:
            chunks_levels.append(cur)
            cur, cur_n = [], 0
        cur.append(li)
        cur_n += len(nodes)
    if cur:
        chunks_levels.append(cur)
    C = len(chunks_levels)

    # row coordinate space: rows 0..511 = input nodes (orig order),
    # then computed chunk c occupies rows 512 + c*128 .. +128 (padded).
    NROW = N_IN + C * CH
    row_of = np.full(N_NODES, -1, np.int64)
    row_of[:N_IN] = np.arange(N_IN)
    node_of_row = np.full(NROW, -1, np.int64)
    node_of_row[:N_IN] = np.arange(N_IN)
    chunk_levels_rows = []  # per chunk: list of (r0, r1) row ranges per level
    for c, lis in enumerate(chunks_levels):
        base = N_IN + c * CH
        off = 0
        lvls = []
        for li in lis:
            nodes = lev_nodes[li]
            r0 = off
            for nd in nodes:
                row_of[nd] = base + off
                node_of_row[base + off] = nd
                off += 1
            lvls.append((r0, off))
        chunk_levels_rows.append(lvls)

    # dense transposed weights in row coords: WT[src_row, dst_row]
    WT = np.zeros((NROW, NROW), np.float32)
    np.add.at(WT, (row_of[edge_src], row_of[edge_dst]), edge_w)

    # bias per chunk column [128, C]; padded rows get 0
    bias128 = np.zeros((CH, C), np.float32)
    for c in range(C):
        rows = node_of_row[N_IN + c * CH : N_IN + (c + 1) * CH]
        valid = rows >= 0
        bias128[valid, c] = b[rows[valid]]

    # level of each row within a computed chunk (for bd split)
    # chunk c's last level rows: chunk_levels_rows[c][-1]

    cols = []
    col = 0
    chunk_meta = []
    for t in range(C):
        dst0 = N_IN + t * CH
        D = WT[:, dst0 : dst0 + CH]
        old = []  # (coloff, src_tile)
        bd_pre = None
        bd_last = None
        nsrc = NCH_IN + t  # source tiles 0..nsrc-1
        for k in range(nsrc):
            blk = D[k * CH : (k + 1) * CH, :]
            if t >= 1 and k == nsrc - 1:
                # previous computed chunk: split entries by src level
                lr0, lr1 = chunk_levels_rows[t - 1][-1]
                pre = blk.copy()
                pre[lr0:lr1, :] = 0.0
                last = blk.copy()
                last[:lr0, :] = 0.0
                last[lr1:, :] = 0.0
                if pre.any():
                    cols.append(pre)
                    bd_pre = (col, k)
                    col += CH
                assert last.any(), "bd_last must be nonzero"
                cols.append(last)
                bd_last = (col, k)
                col += CH
            else:
                if not blk.any():
                    continue
                cols.append(blk)
                old.append((col, k))
                col += CH
        # in-chunk gathers per level gi >= 1
        Din = WT[dst0 : dst0 + CH, dst0 : dst0 + CH]
        gth = []
        lvls = chunk_levels_rows[t]
        for gi in range(1, len(lvls)):
            r0, r1 = lvls[gi]
            blk = np.zeros((CH, CH), np.float32)
            blk[:, r0:r1] = Din[:, r0:r1]
            assert blk.any(), "in-chunk gather must be nonzero"
            cols.append(blk)
            gth.append((col,))
            col += CH
        chunk_meta.append(dict(old=old, bd_pre=bd_pre, bd_last=bd_last, gth=gth))

    wflat = np.ascontiguousarray(np.concatenate(cols, axis=1))  # [128, col]
    out_rows = row_of[np.arange(N_NODES - N_OUT, N_NODES)] - N_IN

    return dict(
        C=C,
        chunk_levels_rows=chunk_levels_rows,
        chunk_meta=chunk_meta,
        wflat=wflat,
        bias128=bias128,
        out_rows=out_rows,
        row_of=row_of,
    )


def _emulate(prep, xT):
    """Numpy emulation of the exact block scheme (per core). xT: [512, B]."""
    B = xT.shape[1]
    C = prep["C"]
    wflat = prep["wflat"]
    bias = prep["bias128"]
    NROW = N_IN + C * CH
    aT = np.zeros((NROW, B), np.float32)
    aT[:N_IN] = xT
    for t in range(C):
        meta = prep["chunk_meta"][t]
        dst0 = N_IN + t * CH
        psum = np.zeros((CH, B), np.float32)
        mms = list(meta["old"])
        if meta["bd_pre"]:
            mms.append(meta["bd_pre"])
        if meta["bd_last"]:
            mms.append(meta["bd_last"])
        for coloff, k in mms:
            blk = wflat[:, coloff : coloff + CH]
            psum += blk.T @ aT[k * CH : (k + 1) * CH]
        lvls = prep["chunk_levels_rows"][t]
        for gi, (r0, r1) in enumerate(lvls):
            if gi > 0:
                (coloff,) = meta["gth"][gi - 1]
                blk = wflat[:, coloff : coloff + CH]
                psum += blk.T @ aT[dst0 : dst0 + CH]
            # full-tile tanh (idempotent rewrite like the kernel)
            aT[dst0 : dst0 + CH] = np.tanh(psum + bias[:, t : t + 1])
    return aT[N_IN:]  # [C*128, B]


def _build_program(prep):
    """Build the Bass/Tile program (identical for all 8 cores)."""
    import concourse.bacc as bacc
    import concourse.tile as tile
    from concourse import mybir

    f32 = mybir.dt.float32
    f16 = mybir.dt.float16
    nc = bacc.Bacc(
        "TRN2",
        target_bir_lowering=False,
        debug=False,
        enable_asserts=False,
        num_devices=NCORES,
    )
    C = prep["C"]
    wflat = prep["wflat"]
    F = wflat.shape[1]
    meta = prep["chunk_meta"]
    lvls_all = prep["chunk_levels_rows"]

    # xT pre-laid-out host-side in SBUF layout: [128, n_in_chunks * batch]
    xT_d = nc.dram_tensor(
        "xT", [CH, NCH_IN * BCORE], f16, kind="ExternalInput"
    ).ap()
    w_d = nc.dram_tensor("wflat", [CH, F], f16, kind="ExternalInput").ap()
    b_d = nc.dram_tensor("bias", [CH, C], f32, kind="ExternalInput").ap()
    out_d = nc.dram_tensor("outT", [C * CH, BCORE], f16, kind="ExternalOutput").ap()

    # per-chunk W tile span (cols) within wflat
    chunk_w0, chunk_w1 = [], []
    for t in range(C):
        offs = [c for c, _ in meta[t]["old"]]
        if meta[t]["bd_pre"]:
            offs.append(meta[t]["bd_pre"][0])
        if meta[t]["bd_last"]:
            offs.append(meta[t]["bd_last"][0])
        offs += [g[0] for g in meta[t]["gth"]]
        chunk_w0.append(min(offs))
        chunk_w1.append(max(offs) + CH)

    with tile.TileContext(nc) as tc:
        with (
            tc.tile_pool(name="aT", bufs=1) as aT_pool,
            tc.tile_pool(name="wpool", bufs=4) as w_pool,
            tc.tile_pool(name="small", bufs=1) as small_pool,
            tc.tile_pool(name="psum", bufs=3, space="PSUM") as psum_pool,
        ):
            # input activations: one wide tile, single DMA
            xTall = aT_pool.tile([CH, NCH_IN * BCORE], f16, tag="xTall")
            aTc = [
                aT_pool.tile([CH, BCORE], f16, tag=f"aT{c}", name=f"aT{c}")
                for c in range(C)
            ]
            bias_t = small_pool.tile([CH, C], f32, tag="bias")
            scratch = small_pool.tile([CH, 1], f32, tag="scratch")

            def rhs(src_tile, lo, hi):
                if src_tile < NCH_IN:
                    base = src_tile * BCORE
                    return xTall[:, base + lo : base + hi]
                return aTc[src_tile - NCH_IN][:, lo:hi]

            w_tiles = [None] * C

            def wdma(t):
                wid = chunk_w1[t] - chunk_w0[t]
                w_tiles[t] = w_pool.tile([CH, wid], f16, tag="w", name=f"w{t}")
                if t == 0:
                    # split so the first matmuls wait only on the off-diag
                    # blocks, not the whole span
                    split = (
                        max(c for c, _ in meta[0]["old"]) + CH - chunk_w0[0]
                    )
                    nc.sync.dma_start(
                        out=w_tiles[0][:, :split],
                        in_=w_d[:, chunk_w0[0] : chunk_w0[0] + split],
                    )
                    if split < wid:
                        nc.sync.dma_start(
                            out=w_tiles[0][:, split:],
                            in_=w_d[:, chunk_w0[0] + split : chunk_w1[0]],
                        )
                    return
                nc.sync.dma_start(
                    out=w_tiles[t][:], in_=w_d[:, chunk_w0[t] : chunk_w1[t]]
                )

            # --- matmul bookkeeping: start/stop flags -------------------
            n_mms = [
                len(meta[t]["old"])
                + (1 if meta[t]["bd_pre"] else 0)
                + (1 if meta[t]["bd_last"] else 0)
                + len(meta[t]["gth"])
                for t in range(C)
            ]
            cnt_mm = [0] * C
            psum_t = [None] * C

            def get_psum(t):
                if psum_t[t] is None:
                    psum_t[t] = psum_pool.tile(
                        [CH, BCORE], f32, tag="psum", name=f"psum{t}"
                    )
                return psum_t[t]

            def emit_mm(t, coloff, src_tile):
                ps = get_psum(t)
                first = cnt_mm[t] == 0
                last = cnt_mm[t] == n_mms[t] - 1
                a = coloff - chunk_w0[t]
                nc.tensor.matmul(
                    ps[:, :],
                    w_tiles[t][:, a : a + CH],
                    rhs(src_tile, 0, BCORE),
                    start=first,
                    stop=last,
                )
                cnt_mm[t] += 1

            def emit_act(t, gi):
                nc.scalar.activation(
                    aTc[t][:, :],
                    psum_t[t][:, :],
                    mybir.ActivationFunctionType.Tanh,
                    bias=bias_t[:, t : t + 1],
                )

            # --- prologue ----------------------------------------------
            wdma(0)
            # xT: one contiguous DMA (host already produced SBUF layout)
            nc.sync.dma_start(out=xTall[:], in_=xT_d[:, :])
            nc.sync.dma_start(out=bias_t[:], in_=b_d[:])
            # preload the tanh table during the prologue
            nc.scalar.activation(
                scratch[:], bias_t[:, 0:1], mybir.ActivationFunctionType.Tanh
            )
            wdma(1)
            wdma(2)
            # chunk 0 full accumulation
            get_psum(0)
            get_psum(1)
            for coloff, k in meta[0]["old"]:
                emit_mm(0, coloff, k)

            # --- global filler schedule --------------------------------
            # Each dst chunk u's "old" mms may run during the walks of
            # chunks u-2..u-1 (psum/w-tile window), as soon as their src
            # tile is final. Spread them least-loaded so no level slot's
            # filler work overflows its act window.
            slot_of = []  # (t, gi)
            first_slot = []
            for t in range(C):
                first_slot.append(len(slot_of))
                for gi in range(len(lvls_all[t])):
                    slot_of.append((t, gi))
            n_slots_total = len(slot_of)
            assign = [[] for _ in range(n_slots_total)]
            load = [0.0] * n_slots_total
            # pin bd_pre load (emitted in slot (u-1, last)):
            for u in range(1, C):
                if meta[u]["bd_pre"]:
                    load[first_slot[u] - 1] += 1.0
            items = []
            for u in range(1, C):
                for coloff, k in meta[u]["old"]:
                    lo = first_slot[max(u - 2, 0)]
                    if k >= NCH_IN:
                        lo = max(lo, first_slot[k - NCH_IN + 1])
                    hi = first_slot[u] - 1
                    items.append((hi, lo, u, coloff, k))
            items.sort()
            for hi, lo, u, coloff, k in items:
                s = min(range(lo, hi + 1), key=lambda x: (load[x], x))
                assign[s].append((u, coloff, k))
                load[s] += 1.0

            # --- main walk ---------------------------------------------
            for t in range(C):
                if t + 3 < C:
                    wdma(t + 3)
                if t + 2 < C:
                    get_psum(t + 2)  # keep psum pool rotation in order
                G = len(lvls_all[t])
                for gi in range(G):
                    # chain mm
                    if gi == 0:
                        if t >= 1:
                            emit_mm(t, meta[t]["bd_last"][0], NCH_IN + t - 1)
                    else:
                        (coloff,) = meta[t]["gth"][gi - 1]
                        emit_mm(t, coloff, NCH_IN + t)
                    if gi == G - 1 and t + 1 < C and meta[t + 1]["bd_pre"]:
                        # rides b2b behind the chain mm (same sem, no stall)
                        emit_mm(t + 1, meta[t + 1]["bd_pre"][0], NCH_IN + t)
                    emit_act(t, gi)
                    # globally scheduled fillers run during this level's act
                    for u, coloff, k in assign[first_slot[t] + gi]:
                        emit_mm(u, coloff, k)
                nc.sync.dma_start(
                    out=out_d[t * CH : (t + 1) * CH, :], in_=aTc[t][:]
                )

    nc.compile()
    return nc


def _make_in_maps(prep, x):
    x = np.asarray(x, dtype=np.float32)
    in_maps = []
    wflat16 = prep["wflat"].astype(np.float16)
    for r in range(NCORES):
        xr = x[r * BCORE : (r + 1) * BCORE]  # [256, 512]
        xT = xr.T.astype(np.float16)  # [512, 256]
        # SBUF layout: partition p, free = chunk-major: [128, 4*256]
        xT = np.ascontiguousarray(
            xT.reshape(NCH_IN, CH, BCORE).transpose(1, 0, 2).reshape(
                CH, NCH_IN * BCORE
            )
        )
        in_maps.append({"xT": xT, "wflat": wflat16, "bias": prep["bias128"]})
    return in_maps


def _assemble(prep, results):
    out_rows = prep["out_rows"]
    y = np.empty((BATCH, N_OUT), np.float32)
    for r in range(NCORES):
        outT = results[r]["outT"].astype(np.float32)
        y[r * BCORE : (r + 1) * BCORE, :] = outT[out_rows, :].T
    return y


def kernel(x, edge_w, b, edge_src, edge_dst, n_out, _trace=False):
    n_out = int(n_out)
    assert n_out == N_OUT, f"hardcoded for n_out={N_OUT}, got {n_out}"
    x = np.asarray(x, dtype=np.float32)
    assert x.shape == (BATCH, N_IN)

    from concourse.bass_utils import run_bass_kernel_spmd

    prep = _prep(edge_src, edge_dst, edge_w, b)
    nc = _build_program(prep)
    in_maps = _make_in_maps(prep, x)
    res = run_bass_kernel_spmd(
        nc, in_maps, core_ids=list(range(NCORES)), trace=_trace
    )
    y = _assemble(prep, res.results)
    if _trace:
        kernel._last_exec_time_ns = res.exec_time_ns
        kernel._last_results = res
    return y


if __name__ == "__main__":
    # host-side emulation check against the jax reference
    sys.path.insert(0, "/root/problem")
    import os

    os.environ.setdefault("JAX_PLATFORMS", "cpu")
    import reference

    inputs = {k: np.asarray(v) for k, v in reference.setup_inputs().items()}
    prep = _prep(
        inputs["edge_src"], inputs["edge_dst"], inputs["edge_w"], inputs["b"]
    )
    print(
        f"C={prep['C']} chunks, F={prep['wflat'].shape[1]} cols, "
        f"levels/chunk={[len(l) for l in prep['chunk_levels_rows']]}"
    )
    expected = np.asarray(reference.reference(**reference.setup_inputs()))
    xT = inputs["x"][:8].T.astype(np.float32)
    aT = _emulate(prep, xT)
    got = aT[prep["out_rows"], :].T
    err = np.abs(got - expected[:8]).max()
    rel = err / np.abs(expected[:8]).max()
    print(f"emulation absmax err {err:.3e}  rel {rel:.3e}")
